# revision 6
# baseline (speedup 1.0000x reference)
"""BERT self-attention (flash-style) Trainium2 Bass kernel.

Full inputs -> full output. Shards data-parallel over batch: batch element i
runs on NeuronCore i (B == 8 == n_cores), no collectives.

Host-side prep (cheap numpy): transpose x / Wqkv / out_w into e-major layouts
(lhsT/rhs contraction-major), permute Wqkv columns into head-pair order
[q0|k0|q1|k1|...|q5|k5|v] so the weights for the first score matmuls arrive
first, fold 1/sqrt(d) into the q block, turn the key-padding mask into an
additive exp bias (0 / -30000) and the query mask into a 0/1 multiplier.

On-chip per core (S=1024, E=768, H=12, D=64):
  qkT[p]: head-pair p of q (slots 0-5) / k (slots 6-11) with head dims on
          partitions, produced pairwise so scores can start early.
  scores: scoresT[sk,sq] per head; the two heads of a pair run concurrently
          on disjoint PE row-groups (tile_position (0,0)/(64,0), K=64).
  exp:    ScalarE Exp with the key-mask as per-partition bias; ~128us of
          serial ScalarE work is the pipeline pacer -- the emission order
          keeps its input queue nonempty from ~8us on.
  ctx:    per (head, sq-half) PSUM accumulation over sk chunks; lhsT is v
          with a ones column appended so PSUM picks up the softmax
          denominator for free. Even heads: [v(64)|ones] -> ctx rows 0:64,
          den row 64. Odd heads: [pad(63)|ones|v(64)] (M=128, costs nothing:
          matmul time is N columns) -> ctx rows 64:128, den row 63, so the
          normalized output lands at the right partitions with no shift.
  norm:   reciprocal_approx_fast on the den row (DVE, ~5x faster than the
          microcoded reciprocal), DMA partition-broadcast of the reciprocal
          row, one DVE multiply psum*recip -> ctxT bf16.
  out:    ctxT.T @ out_w.T per s-chunk, query mask as per-partition scalar.
"""

import sys

if "/opt/trn_rl_repo" not in sys.path:
    sys.path.insert(0, "/opt/trn_rl_repo")

import numpy as np
import ml_dtypes

import concourse.bass as bass
import concourse.bacc as bacc
import concourse.tile as tile
from concourse import mybir
from concourse.bass_utils import run_bass_kernel_spmd

B, S, E, H = 8, 1024, 768, 12
D = E // H            # 64
NP = 128              # SBUF/PSUM partitions
EC = E // NP          # 6 e-chunks (contraction chunks)
SC = S // NP          # 8 sequence chunks
NPAIR = H // 2        # 6 head pairs
SLOT = 65 + 128       # vsb columns per pair: even head 65, odd head 128
VP = NPAIR * SLOT     # 1158 vsb columns per s-chunk
VOFF = 2 * E          # v block column offset in the permuted wqkv (1536)
BF16 = mybir.dt.bfloat16
F32 = mybir.dt.float32
EXP = mybir.ActivationFunctionType.Exp
MASK_NEG = -30000.0


def _body(tc, xt, wqk, bqk, wot, bo, kb, qm, out, with_bias):
    nc = tc.nc

    with tc.tile_pool(name="const", bufs=1) as const:
        # ---- persistent SBUF state -------------------------------------
        # DMA priority order: xt chunks and the pair-0 q/k weight blocks
        # first (the first score matmul needs exactly those), then the
        # remaining qk pairs, the v block, out weights last.
        xt_k, wq_k, wo_k = [], [], []
        for k in range(EC):
            xk = const.tile([NP, S], BF16, name=f"xt{k}")
            nc.sync.dma_start(out=xk, in_=xt[k * NP:(k + 1) * NP, :])
            xt_k.append(xk)
            wq_k.append(const.tile([NP, 3 * E], BF16, name=f"wq{k}"))
        for p in range(NPAIR):
            for k in range(EC):
                nc.sync.dma_start(
                    out=wq_k[k][:, p * 256:(p + 1) * 256],
                    in_=wqk[k * NP:(k + 1) * NP, p * 256:(p + 1) * 256],
                )
        kb_sb = const.tile([NP, SC], F32)          # key mask bias, col c = s-chunk c
        nc.sync.dma_start(out=kb_sb, in_=kb.rearrange("(c p) -> p c", p=NP))
        qm_sb = const.tile([NP, SC], F32)          # query mask 0/1, col m = s-chunk m
        nc.sync.dma_start(out=qm_sb, in_=qm.rearrange("(c p) -> p c", p=NP))
        for k in range(EC):
            nc.sync.dma_start(
                out=wq_k[k][:, VOFF:3 * E],
                in_=wqk[k * NP:(k + 1) * NP, VOFF:3 * E],
            )
        for k in range(EC):
            ok = const.tile([NP, E], BF16, name=f"wo{k}")
            nc.sync.dma_start(out=ok, in_=wot[k * NP:(k + 1) * NP, :])
            wo_k.append(ok)
        if with_bias:
            bq_sb = const.tile([NP, 2 * H], F32)   # qk bias, col j = perm f-chunk j
            nc.sync.dma_start(
                out=bq_sb,
                in_=bass.AP(tensor=bqk, offset=0, ap=[[1, NP], [NP, 2 * H]]),
            )
            bvcol = const.tile([NP, H], F32)       # v bias: rows 0:64 and 64:128
            for lo in (0, 64):
                nc.sync.dma_start(
                    out=bvcol[lo:lo + 64, :],
                    in_=bass.AP(tensor=bqk, offset=VOFF, ap=[[1, 64], [64, H]]),
                )
            bo_bc = const.tile([NP, E], F32)       # out bias broadcast
            nc.sync.dma_start(
                out=bo_bc, in_=bass.AP(tensor=bo, offset=0, ap=[[0, NP], [1, E]])
            )

        _compute(tc, nc, with_bias,
                 xt_k, wq_k, wo_k, kb_sb, qm_sb, out,
                 bq_sb if with_bias else None,
                 bvcol if with_bias else None,
                 bo_bc if with_bias else None)


def _compute(tc, nc, with_bias, xt_k, wq_k, wo_k, kb_sb, qm_sb, out,
             bq_sb, bvcol, bo_bc):
    with tc.tile_pool(name="work", bufs=1) as work:
        # qT/kT: [128, j, s] bf16; partition = dim within head pair.
        # j=0..5 q pairs (heads 2j,2j+1 at partitions 0-63 / 64-127),
        # j=6..11 k pairs.
        qkT = work.tile([NP, H, S], BF16)
        # v (+ per-head denominator columns): s-chunk on partitions. Pair p
        # occupies SLOT columns: even head [v(64)|ones], odd head
        # [pad(63)|ones|v(64)]; pad/ones come from a single memset(1.0).
        vsb = work.tile([NP, SC, VP], BF16)
        # ctx.T: pair j -> partitions 0:64 head 2j, 64:128 head 2j+1.
        ctxT = work.tile([NP, EC, S], BF16)

        with tc.tile_pool(name="norm", bufs=3) as norm_pool, \
             tc.tile_pool(name="exps", bufs=34) as exps, \
             tc.tile_pool(name="osb", bufs=3) as outp, \
             tc.tile_pool(name="dscr", bufs=1, space="DRAM") as dpool, \
             tc.tile_pool(name="ps_sc", bufs=3, space="PSUM") as ps_sc, \
             tc.tile_pool(name="ps_ctx", bufs=2, space="PSUM") as ps_ctx:

            # DRAM scratch for the softmax-denominator reciprocal: DMA can
            # re-stripe arbitrarily through DRAM (on-chip partition re-striping
            # is illegal), so den rows bounce DRAM-wards to become [128, 4]
            # columns for a cheap batched reciprocal, and the reciprocal row
            # bounces back via a 0-stride partition-broadcast read. 8 slots =
            # two pairs of pipelining depth.
            NSLOT = 8
            dscr = dpool.tile([NSLOT, 512], F32)
            rscr = dpool.tile([NSLOT, 512], BF16)

            for m in range(SC):
                nc.gpsimd.memset(vsb[:, m, :], 1.0)

            def emit_v(m):
                pv = ps_sc.tile([NP, S], F32, tag="sc")
                for k in range(EC):
                    st, sp = (k == 0), (k == EC - 1)
                    nc.tensor.matmul(
                        pv[:, 0:512],
                        lhsT=xt_k[k][:, m * NP:(m + 1) * NP],
                        rhs=wq_k[k][:, VOFF:VOFF + 512],
                        start=st, stop=sp,
                    )
                    nc.tensor.matmul(
                        pv[:, 512:768],
                        lhsT=xt_k[k][:, m * NP:(m + 1) * NP],
                        rhs=wq_k[k][:, VOFF + 512:VOFF + 768],
                        start=st, stop=sp,
                    )
                # scatter heads into their vsb slots: even head of pair p at
                # cols p*SLOT..+64, odd head at p*SLOT+129..+193.
                v_pairs = vsb[:, m, :].rearrange("p (pr s) -> p pr s", s=SLOT)
                pv_pairs = pv[:, 0:768].rearrange("p (pr s) -> p pr s", s=2 * D)
                nc.vector.tensor_copy(
                    out=v_pairs[:, :, 0:D], in_=pv_pairs[:, :, 0:D])
                nc.vector.tensor_copy(
                    out=v_pairs[:, :, D + 65:SLOT], in_=pv_pairs[:, :, D:2 * D])

            def emit_qkT(j):
                # permuted wqkv layout: q pair j at cols j*256, k pair j-6 at
                # (j-6)*256+128
                off = j * 256 if j < NPAIR else (j - NPAIR) * 256 + 128
                pq = ps_sc.tile([NP, S], F32, tag="sc")
                for k in range(EC):
                    st, sp = (k == 0), (k == EC - 1)
                    for n in (0, 512):
                        nc.tensor.matmul(
                            pq[:, n:n + 512],
                            lhsT=wq_k[k][:, off:off + NP],
                            rhs=xt_k[k][:, n:n + 512],
                            start=st, stop=sp,
                        )
                nc.vector.tensor_copy(out=qkT[:, j, :], in_=pq)
                if with_bias:
                    jperm = 2 * j if j < NPAIR else 2 * (j - NPAIR) + 1
                    nc.vector.tensor_scalar_add(
                        out=qkT[:, j, :], in0=qkT[:, j, :],
                        scalar1=bq_sb[:, jperm:jperm + 1],
                    )

            pair_exps = {}

            def emit_scores(p):
                eA, eB = [], []
                for c in range(SC):
                    psA = ps_sc.tile([NP, S], F32, tag="sc")
                    psB = ps_sc.tile([NP, S], F32, tag="sc")
                    for n in (0, 512):
                        nc.tensor.matmul(
                            psA[:, n:n + 512],
                            lhsT=qkT[0:64, NPAIR + p, c * NP:(c + 1) * NP],
                            rhs=qkT[0:64, p, n:n + 512],
                            start=True, stop=True, tile_position=(0, 0),
                        )
                        nc.tensor.matmul(
                            psB[:, n:n + 512],
                            lhsT=qkT[64:128, NPAIR + p, c * NP:(c + 1) * NP],
                            rhs=qkT[64:128, p, n:n + 512],
                            start=True, stop=True, tile_position=(64, 0),
                        )
                    tA = exps.tile([NP, S], BF16, tag="exp")
                    tB = exps.tile([NP, S], BF16, tag="exp")
                    nc.scalar.activation(tA, psA, EXP, bias=kb_sb[:, c:c + 1])
                    nc.scalar.activation(tB, psB, EXP, bias=kb_sb[:, c:c + 1])
                    eA.append(tA)
                    eB.append(tB)
                pair_exps[p] = (eA, eB)

            def emit_ctx(p):
                eA, eB = pair_exps.pop(p)
                for hi, elist in ((0, eA), (1, eB)):
                    h = 2 * p + hi
                    # even head: [v|ones] M=65, den row 64, ctx rows 0:64.
                    # odd head: [pad(32)|ones|pad(31)|v] M=128, den row 32
                    # (engine partition offsets must be 32-aligned), ctx rows
                    # 64:128 (lands at the partitions ctxT wants).
                    if hi == 0:
                        sl0, msz, dr, lo = p * SLOT, 65, 64, 0
                    else:
                        sl0, msz, dr, lo = p * SLOT + 65, 128, 32, 64
                    for half in (0, 1):
                        n0 = half * 512
                        pc = ps_ctx.tile([NP, 512], F32, tag="ctx")
                        for c in range(SC):
                            nc.tensor.matmul(
                                pc[0:msz, :],
                                lhsT=vsb[:, c, sl0:sl0 + msz],
                                rhs=elist[c][:, n0:n0 + 512],
                                start=(c == 0), stop=(c == SC - 1),
                            )
                        # softmax denominator: den row -> DRAM -> [128, 4]
                        # columns, batched reciprocal (128 lanes x 4 elems vs
                        # 1 lane x 512 for a row-wise reciprocal), back to
                        # DRAM, 0-stride broadcast read across the 64 ctx
                        # partitions, one psum*recip multiply.
                        slot = (2 * hi + half + 4 * p) % NSLOT
                        dn = norm_pool.tile([NP, 512], F32, tag="denrow")
                        nc.vector.tensor_copy(
                            out=dn[dr:dr + 1, :], in_=pc[dr:dr + 1, :])
                        nc.sync.dma_start(
                            out=dscr[slot, :], in_=dn[dr:dr + 1, :])
                        cols = norm_pool.tile([NP, 4], F32, tag="cols")
                        nc.sync.dma_start(
                            out=cols,
                            in_=dscr[slot, :].rearrange("(p c) -> p c", c=4))
                        rcols = norm_pool.tile([NP, 4], BF16, tag="rcols")
                        with nc.allow_low_precision(reason="softmax denom recip"):
                            nc.vector.reciprocal(rcols, cols)
                        nc.sync.dma_start(
                            out=rscr[slot, :].rearrange("(p c) -> p c", c=4),
                            in_=rcols)
                        rb = norm_pool.tile([NP, 512], BF16, tag="rbc")
                        nc.sync.dma_start(
                            out=rb[lo:lo + 64, :],
                            in_=bass.AP(
                                tensor=rscr.tensor,
                                offset=rscr.offset + slot * 512,
                                ap=[[0, 64], [1, 512]],
                            ),
                        )
                        dst = ctxT[lo:lo + 64, p, n0:n0 + 512]
                        nc.vector.tensor_mul(
                            out=dst, in0=pc[lo:lo + 64, :], in1=rb[lo:lo + 64, :],
                        )
                        if with_bias:
                            nc.vector.tensor_scalar_add(
                                out=dst, in0=dst,
                                scalar1=bvcol[lo:lo + 64, h:h + 1],
                            )

            def emit_out(m):
                po = ps_sc.tile([NP, S], F32, tag="sc")
                for j in range(EC):
                    st, sp = (j == 0), (j == EC - 1)
                    nc.tensor.matmul(
                        po[:, 0:512],
                        lhsT=ctxT[:, j, m * NP:(m + 1) * NP],
                        rhs=wo_k[j][:, 0:512],
                        start=st, stop=sp,
                    )
                    nc.tensor.matmul(
                        po[:, 512:768],
                        lhsT=ctxT[:, j, m * NP:(m + 1) * NP],
                        rhs=wo_k[j][:, 512:768],
                        start=st, stop=sp,
                    )
                osb = outp.tile([NP, E], F32, tag="osb")
                nc.vector.tensor_scalar_mul(osb, po[:, 0:E], qm_sb[:, m:m + 1])
                if with_bias:
                    nc.vector.tensor_add(osb, osb, bo_bc)
                nc.sync.dma_start(out=out[m * NP:(m + 1) * NP, :], in_=osb)

            # ---- pipelined emission ------------------------------------
            # ScalarE's exp stream is the pacer; this order starts it after
            # one qk pair + one scores chunk and keeps >=1 pair of scores
            # queued ahead of it for the rest of the kernel.
            emit_qkT(0)
            emit_qkT(NPAIR)
            emit_scores(0)
            emit_qkT(1)
            emit_qkT(NPAIR + 1)
            emit_scores(1)
            for m in range(SC):
                emit_v(m)
            for stage in range(2, NPAIR):
                emit_qkT(stage)
                emit_qkT(NPAIR + stage)
                emit_scores(stage)
                emit_ctx(stage - 2)
            emit_ctx(NPAIR - 2)
            emit_ctx(NPAIR - 1)
            for m in range(SC):
                emit_out(m)


def build_nc(with_bias=True):
    nc = bacc.Bacc()
    xt = nc.dram_tensor("xt", [E, S], BF16, kind="ExternalInput")
    wqk = nc.dram_tensor("wqkvt", [E, 3 * E], BF16, kind="ExternalInput")
    bqk = nc.dram_tensor("bqkv", [3 * E], F32, kind="ExternalInput")
    wot = nc.dram_tensor("wot", [E, E], BF16, kind="ExternalInput")
    bo = nc.dram_tensor("bo", [E], F32, kind="ExternalInput")
    kb = nc.dram_tensor("kbias", [S], F32, kind="ExternalInput")
    qm = nc.dram_tensor("qmask", [S], F32, kind="ExternalInput")
    out = nc.dram_tensor("out", [S, E], F32, kind="ExternalOutput")
    with tile.TileContext(nc) as tc:
        _body(tc, xt, wqk, bqk, wot, bo, kb, qm, out, with_bias)
    nc.compile()
    return nc


def _perm_cols():
    """Column permutation for the fused qkv weight: [q0|k0|q1|k1|...|v]."""
    idx = []
    for p in range(NPAIR):
        idx.extend(range(p * NP, (p + 1) * NP))          # q pair p (scaled)
        idx.extend(range(E + p * NP, E + (p + 1) * NP))  # k pair p
    idx.extend(range(2 * E, 3 * E))                      # v block
    return np.asarray(idx)


def prep_in_maps(x, key_padding_mask, Wqkv_w, Wqkv_b, out_w, out_b):
    bf16 = ml_dtypes.bfloat16
    x = np.asarray(x, np.float32)
    mask = np.asarray(key_padding_mask).astype(bool)
    scale = 1.0 / np.sqrt(np.float32(D))

    wqkvT = np.asarray(Wqkv_w, np.float32).T.copy()      # (E, 3E), e-major
    wqkvT[:, :E] *= scale                                # fold 1/sqrt(d) into Wq
    bqkv = np.asarray(Wqkv_b, np.float32).copy()
    bqkv[:E] *= scale
    perm = _perm_cols()
    wqkvT = wqkvT[:, perm]
    bqkv = bqkv[perm]
    wotT = np.asarray(out_w, np.float32).T.copy()        # (E, E), e-major

    wqkvT = np.ascontiguousarray(wqkvT).astype(bf16)
    wotT = np.ascontiguousarray(wotT).astype(bf16)
    bo_ = np.asarray(out_b, np.float32)

    in_maps = []
    for i in range(B):
        xti = np.ascontiguousarray(x[i].T).astype(bf16)  # (E, S)
        kbias = np.where(mask[i], 0.0, MASK_NEG).astype(np.float32)
        qmask = mask[i].astype(np.float32)
        in_maps.append(
            {
                "xt": xti,
                "wqkvt": wqkvT,
                "bqkv": bqkv,
                "wot": wotT,
                "bo": bo_,
                "kbias": kbias,
                "qmask": qmask,
            }
        )
    return in_maps


_NC_CACHE = {}


def _get_nc(with_bias=True):
    if with_bias not in _NC_CACHE:
        _NC_CACHE[with_bias] = build_nc(with_bias)
    return _NC_CACHE[with_bias]


def kernel(x, key_padding_mask, Wqkv_w, Wqkv_b, out_w, out_b):
    in_maps = prep_in_maps(x, key_padding_mask, Wqkv_w, Wqkv_b, out_w, out_b)
    with_bias = bool(np.any(np.asarray(Wqkv_b) != 0) or np.any(np.asarray(out_b) != 0))
    nc = _get_nc(with_bias)
    res = run_bass_kernel_spmd(nc, in_maps, core_ids=list(range(B)))
    out = np.stack([res.results[i]["out"] for i in range(B)], axis=0)
    return out.astype(np.float32)


if __name__ == "__main__":
    nc = build_nc(with_bias=False)
    print("build ok")


# revision 11
# speedup vs baseline: 1.5216x; 1.5216x over previous
"""BERT self-attention (flash-style) Trainium2 Bass kernel.

Full inputs -> full output. Shards data-parallel over batch: batch element i
runs on NeuronCore i (B == 8 == n_cores), no collectives.

Host-side prep (cheap numpy): transpose x / Wqkv / out_w into e-major layouts
(lhsT/rhs contraction-major), permute Wqkv columns into head-pair order
[q0|k0|q1|k1|...|q5|k5|v] so the weights for the first score matmuls arrive
first, fold 1/sqrt(d) into the q block, turn the key-padding mask into an
additive exp bias (0 / -30000) and the query mask into a 0/1 multiplier.

On-chip per core (S=1024, E=768, H=12, D=64):
  qkT[p]: head-pair p of q (slots 0-5) / k (slots 6-11) with head dims on
          partitions, produced pairwise so scores can start early.
  scores: scoresT[sk,sq] per head; the two heads of a pair run concurrently
          on disjoint PE row-groups (tile_position (0,0)/(64,0), K=64).
  exp:    ScalarE Exp with the key-mask as per-partition bias; ~128us of
          serial ScalarE work is the pipeline pacer -- the emission order
          keeps its input queue nonempty from ~8us on.
  ctx:    per (head, sq-half) PSUM accumulation over sk chunks; lhsT is v
          with a ones column appended so PSUM picks up the softmax
          denominator for free. Even heads: [v(64)|ones] -> ctx rows 0:64,
          den row 64. Odd heads: [pad(63)|ones|v(64)] (M=128, costs nothing:
          matmul time is N columns) -> ctx rows 64:128, den row 63, so the
          normalized output lands at the right partitions with no shift.
  norm:   reciprocal_approx_fast on the den row (DVE, ~5x faster than the
          microcoded reciprocal), DMA partition-broadcast of the reciprocal
          row, one DVE multiply psum*recip -> ctxT bf16.
  out:    ctxT.T @ out_w.T per s-chunk, query mask as per-partition scalar.
"""

import sys

if "/opt/trn_rl_repo" not in sys.path:
    sys.path.insert(0, "/opt/trn_rl_repo")

import numpy as np
import ml_dtypes

import concourse.bass as bass
import concourse.bacc as bacc
import concourse.tile as tile
from concourse import mybir
from concourse.bass_utils import run_bass_kernel_spmd

B, S, E, H = 8, 1024, 768, 12
D = E // H            # 64
NP = 128              # SBUF/PSUM partitions
EC = E // NP          # 6 e-chunks (contraction chunks)
SC = S // NP          # 8 sequence chunks
NPAIR = H // 2        # 6 head pairs
SLOT = 65 + 128       # vsb columns per pair: even head 65, odd head 128
VP = NPAIR * SLOT     # 1158 vsb columns per s-chunk
VOFF = 2 * E          # v block column offset in the permuted wqkv (1536)
BF16 = mybir.dt.bfloat16
F32 = mybir.dt.float32
EXP = mybir.ActivationFunctionType.Exp
MASK_NEG = -30000.0


def _body(tc, xt, wqk, bqk, wot, bo, kb, qm, out, with_bias):
    nc = tc.nc

    with tc.tile_pool(name="const", bufs=1) as const:
        # ---- persistent SBUF state -------------------------------------
        # DMA priority order (the SP engine issues DMAs serially at ~0.6us
        # each, so order = latency): the tiny mask vectors first (the exp
        # activations need kb as bias), then xt chunks and the pair-0 q/k
        # weight block (the first score matmul needs exactly those), then
        # the remaining weights.
        kb_sb = const.tile([NP, SC], F32)          # key mask bias, col c = s-chunk c
        nc.sync.dma_start(out=kb_sb, in_=kb.rearrange("(c p) -> p c", p=NP))
        qm_sb = const.tile([NP, SC], F32)          # query mask 0/1, col m = s-chunk m
        nc.sync.dma_start(out=qm_sb, in_=qm.rearrange("(c p) -> p c", p=NP))
        xt_k, wq_k, wo_k = [], [], []
        for k in range(EC):
            xk = const.tile([NP, S], BF16, name=f"xt{k}")
            nc.sync.dma_start(out=xk, in_=xt[k * NP:(k + 1) * NP, :])
            xt_k.append(xk)
            wq_k.append(const.tile([NP, 3 * E], BF16, name=f"wq{k}"))
        for k in range(EC):
            nc.sync.dma_start(
                out=wq_k[k][:, 0:256],
                in_=wqk[k * NP:(k + 1) * NP, 0:256],
            )
        for k in range(EC):
            nc.sync.dma_start(
                out=wq_k[k][:, 256:VOFF],
                in_=wqk[k * NP:(k + 1) * NP, 256:VOFF],
            )
        for k in range(EC):
            nc.sync.dma_start(
                out=wq_k[k][:, VOFF:3 * E],
                in_=wqk[k * NP:(k + 1) * NP, VOFF:3 * E],
            )
        for k in range(EC):
            ok = const.tile([NP, E], BF16, name=f"wo{k}")
            nc.sync.dma_start(out=ok, in_=wot[k * NP:(k + 1) * NP, :])
            wo_k.append(ok)
        if with_bias:
            bq_sb = const.tile([NP, 2 * H], F32)   # qk bias, col j = perm f-chunk j
            nc.sync.dma_start(
                out=bq_sb,
                in_=bass.AP(tensor=bqk, offset=0, ap=[[1, NP], [NP, 2 * H]]),
            )
            bvcol = const.tile([NP, H], F32)       # v bias: rows 0:64 and 64:128
            for lo in (0, 64):
                nc.sync.dma_start(
                    out=bvcol[lo:lo + 64, :],
                    in_=bass.AP(tensor=bqk, offset=VOFF, ap=[[1, 64], [64, H]]),
                )
            bo_bc = const.tile([NP, E], F32)       # out bias broadcast
            nc.sync.dma_start(
                out=bo_bc, in_=bass.AP(tensor=bo, offset=0, ap=[[0, NP], [1, E]])
            )

        _compute(tc, nc, with_bias,
                 xt_k, wq_k, wo_k, kb_sb, qm_sb, out,
                 bq_sb if with_bias else None,
                 bvcol if with_bias else None,
                 bo_bc if with_bias else None)


def _compute(tc, nc, with_bias, xt_k, wq_k, wo_k, kb_sb, qm_sb, out,
             bq_sb, bvcol, bo_bc):
    with tc.tile_pool(name="work", bufs=1) as work:
        # qT/kT: [128, j, s] bf16; partition = dim within head pair.
        # j=0..5 q pairs (heads 2j,2j+1 at partitions 0-63 / 64-127),
        # j=6..11 k pairs.
        qkT = work.tile([NP, H, S], BF16)
        # v (+ per-head denominator columns): s-chunk on partitions. Pair p
        # occupies SLOT columns: even head [v(64)|ones], odd head
        # [pad(63)|ones|v(64)]; pad/ones come from a single memset(1.0).
        vsb = work.tile([NP, SC, VP], BF16)
        # ctx.T: pair j -> partitions 0:64 head 2j, 64:128 head 2j+1.
        ctxT = work.tile([NP, EC, S], BF16)

        with tc.tile_pool(name="norm", bufs=3) as norm_pool, \
             tc.tile_pool(name="exps", bufs=34) as exps, \
             tc.tile_pool(name="osb", bufs=3) as outp, \
             tc.tile_pool(name="dscr", bufs=1, space="DRAM") as dpool, \
             tc.tile_pool(name="ps_sc", bufs=2, space="PSUM") as ps_sc, \
             tc.tile_pool(name="ps_ctx", bufs=4, space="PSUM") as ps_ctx:

            # DRAM scratch for the softmax-denominator reciprocal: DMA can
            # re-stripe arbitrarily through DRAM (on-chip partition
            # re-striping is illegal), so the 4 den rows of a pair bounce
            # DRAM-wards to become [128, 16] columns for one cheap batched
            # reciprocal, and the reciprocal rows bounce back via 0-stride
            # partition-broadcast reads. 2 slots = two pairs in flight.
            dscr = dpool.tile([2, 2048], F32)
            rscr = dpool.tile([2, 2048], BF16)

            for m in range(SC):
                nc.gpsimd.memset(vsb[:, m, :], 1.0)

            def emit_v(m):
                pv = ps_sc.tile([NP, S], F32, tag="sc")
                for k in range(EC):
                    st, sp = (k == 0), (k == EC - 1)
                    nc.tensor.matmul(
                        pv[:, 0:512],
                        lhsT=xt_k[k][:, m * NP:(m + 1) * NP],
                        rhs=wq_k[k][:, VOFF:VOFF + 512],
                        start=st, stop=sp,
                    )
                    nc.tensor.matmul(
                        pv[:, 512:768],
                        lhsT=xt_k[k][:, m * NP:(m + 1) * NP],
                        rhs=wq_k[k][:, VOFF + 512:VOFF + 768],
                        start=st, stop=sp,
                    )
                # scatter heads into their vsb slots: even head of pair p at
                # cols p*SLOT..+64, odd head at p*SLOT+129..+193.
                v_pairs = vsb[:, m, :].rearrange("p (pr s) -> p pr s", s=SLOT)
                pv_pairs = pv[:, 0:768].rearrange("p (pr s) -> p pr s", s=2 * D)
                nc.vector.tensor_copy(
                    out=v_pairs[:, :, 0:D], in_=pv_pairs[:, :, 0:D])
                nc.vector.tensor_copy(
                    out=v_pairs[:, :, D + 65:SLOT], in_=pv_pairs[:, :, D:2 * D])

            def emit_qkT(j):
                # permuted wqkv layout: q pair j at cols j*256, k pair j-6 at
                # (j-6)*256+128
                off = j * 256 if j < NPAIR else (j - NPAIR) * 256 + 128
                pq = ps_sc.tile([NP, S], F32, tag="sc")
                for k in range(EC):
                    st, sp = (k == 0), (k == EC - 1)
                    for n in (0, 512):
                        nc.tensor.matmul(
                            pq[:, n:n + 512],
                            lhsT=wq_k[k][:, off:off + NP],
                            rhs=xt_k[k][:, n:n + 512],
                            start=st, stop=sp,
                        )
                nc.vector.tensor_copy(out=qkT[:, j, :], in_=pq)
                if with_bias:
                    jperm = 2 * j if j < NPAIR else 2 * (j - NPAIR) + 1
                    nc.vector.tensor_scalar_add(
                        out=qkT[:, j, :], in0=qkT[:, j, :],
                        scalar1=bq_sb[:, jperm:jperm + 1],
                    )

            pair_exps = {}

            def emit_scores(p):
                eA, eB = [], []
                for c in range(SC):
                    psA = ps_sc.tile([NP, S], F32, tag="sc")
                    psB = ps_sc.tile([NP, S], F32, tag="sc")
                    for n in (0, 512):
                        nc.tensor.matmul(
                            psA[:, n:n + 512],
                            lhsT=qkT[0:64, NPAIR + p, c * NP:(c + 1) * NP],
                            rhs=qkT[0:64, p, n:n + 512],
                            start=True, stop=True, tile_position=(0, 0),
                        )
                        nc.tensor.matmul(
                            psB[:, n:n + 512],
                            lhsT=qkT[64:128, NPAIR + p, c * NP:(c + 1) * NP],
                            rhs=qkT[64:128, p, n:n + 512],
                            start=True, stop=True, tile_position=(64, 0),
                        )
                    tA = exps.tile([NP, S], BF16, tag="exp")
                    tB = exps.tile([NP, S], BF16, tag="exp")
                    nc.scalar.activation(tA, psA, EXP, bias=kb_sb[:, c:c + 1])
                    nc.scalar.activation(tB, psB, EXP, bias=kb_sb[:, c:c + 1])
                    eA.append(tA)
                    eB.append(tB)
                pair_exps[p] = (eA, eB)

            def emit_ctx(p):
                eA, eB = pair_exps.pop(p)
                slot = p % 2
                # even head: [v|ones] M=65, den row 64, ctx rows 0:64.
                # odd head: [pad(32)|ones|pad(31)|v] M=128, den row 32
                # (engine partition offsets must be 32-aligned), ctx rows
                # 64:128 (lands at the partitions ctxT wants).
                # The pair's 4 denominator rows gather into dn rows {64, 32}
                # x cols 0:1024 so ONE DRAM bounce + ONE [128, 16] reciprocal
                # serves all four ctx blocks.
                dn = norm_pool.tile([NP, S], F32, tag="denrow")
                pcs = {}
                for hi, elist in ((0, eA), (1, eB)):
                    sl0, msz, dr = (
                        (p * SLOT, 65, 64) if hi == 0
                        else (p * SLOT + 65, 128, 32)
                    )
                    for half in (0, 1):
                        n0 = half * 512
                        pc = ps_ctx.tile([NP, 512], F32, tag="ctx")
                        pcs[(hi, half)] = pc
                        for c in range(SC):
                            nc.tensor.matmul(
                                pc[0:msz, :],
                                lhsT=vsb[:, c, sl0:sl0 + msz],
                                rhs=elist[c][:, n0:n0 + 512],
                                start=(c == 0), stop=(c == SC - 1),
                            )
                        nc.vector.tensor_copy(
                            out=dn[dr:dr + 1, n0:n0 + 512],
                            in_=pc[dr:dr + 1, :])
                nc.sync.dma_start(out=dscr[slot, 0:1024], in_=dn[64:65, :])
                nc.sync.dma_start(out=dscr[slot, 1024:2048], in_=dn[32:33, :])
                cols = norm_pool.tile([NP, 16], F32, tag="cols")
                nc.sync.dma_start(
                    out=cols, in_=dscr[slot, :].rearrange("(p c) -> p c", c=16))
                rcols = norm_pool.tile([NP, 16], BF16, tag="rcols")
                with nc.allow_low_precision(reason="softmax denom recip"):
                    nc.vector.reciprocal(rcols, cols)
                nc.sync.dma_start(
                    out=rscr[slot, :].rearrange("(p c) -> p c", c=16),
                    in_=rcols)
                rb = norm_pool.tile([NP, S], BF16, tag="rbc")
                for hi in (0, 1):
                    lo = 64 * hi
                    nc.sync.dma_start(
                        out=rb[lo:lo + 64, :],
                        in_=bass.AP(
                            tensor=rscr.tensor,
                            offset=rscr.offset + slot * 2048 + hi * 1024,
                            ap=[[0, 64], [1, 1024]],
                        ),
                    )
                for hi in (0, 1):
                    h = 2 * p + hi
                    lo = 64 * hi
                    for half in (0, 1):
                        n0 = half * 512
                        pc = pcs[(hi, half)]
                        dst = ctxT[lo:lo + 64, p, n0:n0 + 512]
                        nc.vector.tensor_mul(
                            out=dst, in0=pc[lo:lo + 64, :],
                            in1=rb[lo:lo + 64, n0:n0 + 512],
                        )
                        if with_bias:
                            nc.vector.tensor_scalar_add(
                                out=dst, in0=dst,
                                scalar1=bvcol[lo:lo + 64, h:h + 1],
                            )

            def emit_out(m):
                po = ps_sc.tile([NP, S], F32, tag="sc")
                for j in range(EC):
                    st, sp = (j == 0), (j == EC - 1)
                    nc.tensor.matmul(
                        po[:, 0:512],
                        lhsT=ctxT[:, j, m * NP:(m + 1) * NP],
                        rhs=wo_k[j][:, 0:512],
                        start=st, stop=sp,
                    )
                    nc.tensor.matmul(
                        po[:, 512:768],
                        lhsT=ctxT[:, j, m * NP:(m + 1) * NP],
                        rhs=wo_k[j][:, 512:768],
                        start=st, stop=sp,
                    )
                osb = outp.tile([NP, E], F32, tag="osb")
                nc.vector.tensor_scalar_mul(osb, po[:, 0:E], qm_sb[:, m:m + 1])
                if with_bias:
                    nc.vector.tensor_add(osb, osb, bo_bc)
                nc.sync.dma_start(out=out[m * NP:(m + 1) * NP, :], in_=osb)

            # ---- pipelined emission ------------------------------------
            # ScalarE's exp stream is the pacer. scores(0)/scores(1) fill
            # its queue (32 acts = the exps pool, ~36us of work), the v
            # projection runs under that shadow, and from scores(2) on each
            # pair's scores land just as the previous pair's exps drain.
            # scores(2) must come AFTER v: it parks both ps_sc slots
            # waiting for exp-pool space that only ctx(0) (which needs v)
            # can free -- emitting v later deadlocks the slot allocation.
            for stage in range(2):
                emit_qkT(stage)
                emit_qkT(NPAIR + stage)
                emit_scores(stage)
            emit_qkT(2)
            emit_qkT(NPAIR + 2)
            for m in range(SC):
                emit_v(m)
            emit_scores(2)
            emit_ctx(0)
            for stage in range(3, NPAIR):
                emit_qkT(stage)
                emit_qkT(NPAIR + stage)
                emit_scores(stage)
                emit_ctx(stage - 2)
            emit_ctx(NPAIR - 2)
            emit_ctx(NPAIR - 1)
            for m in range(SC):
                emit_out(m)


def build_nc(with_bias=True):
    nc = bacc.Bacc()
    xt = nc.dram_tensor("xt", [E, S], BF16, kind="ExternalInput")
    wqk = nc.dram_tensor("wqkvt", [E, 3 * E], BF16, kind="ExternalInput")
    bqk = nc.dram_tensor("bqkv", [3 * E], F32, kind="ExternalInput")
    wot = nc.dram_tensor("wot", [E, E], BF16, kind="ExternalInput")
    bo = nc.dram_tensor("bo", [E], F32, kind="ExternalInput")
    kb = nc.dram_tensor("kbias", [S], F32, kind="ExternalInput")
    qm = nc.dram_tensor("qmask", [S], F32, kind="ExternalInput")
    out = nc.dram_tensor("out", [S, E], F32, kind="ExternalOutput")
    with tile.TileContext(nc) as tc:
        _body(tc, xt, wqk, bqk, wot, bo, kb, qm, out, with_bias)
    nc.compile()
    return nc


def _perm_cols():
    """Column permutation for the fused qkv weight: [q0|k0|q1|k1|...|v]."""
    idx = []
    for p in range(NPAIR):
        idx.extend(range(p * NP, (p + 1) * NP))          # q pair p (scaled)
        idx.extend(range(E + p * NP, E + (p + 1) * NP))  # k pair p
    idx.extend(range(2 * E, 3 * E))                      # v block
    return np.asarray(idx)


def prep_in_maps(x, key_padding_mask, Wqkv_w, Wqkv_b, out_w, out_b):
    bf16 = ml_dtypes.bfloat16
    x = np.asarray(x, np.float32)
    mask = np.asarray(key_padding_mask).astype(bool)
    scale = 1.0 / np.sqrt(np.float32(D))

    wqkvT = np.asarray(Wqkv_w, np.float32).T.copy()      # (E, 3E), e-major
    wqkvT[:, :E] *= scale                                # fold 1/sqrt(d) into Wq
    bqkv = np.asarray(Wqkv_b, np.float32).copy()
    bqkv[:E] *= scale
    perm = _perm_cols()
    wqkvT = wqkvT[:, perm]
    bqkv = bqkv[perm]
    wotT = np.asarray(out_w, np.float32).T.copy()        # (E, E), e-major

    wqkvT = np.ascontiguousarray(wqkvT).astype(bf16)
    wotT = np.ascontiguousarray(wotT).astype(bf16)
    bo_ = np.asarray(out_b, np.float32)

    in_maps = []
    for i in range(B):
        xti = np.ascontiguousarray(x[i].T).astype(bf16)  # (E, S)
        kbias = np.where(mask[i], 0.0, MASK_NEG).astype(np.float32)
        qmask = mask[i].astype(np.float32)
        in_maps.append(
            {
                "xt": xti,
                "wqkvt": wqkvT,
                "bqkv": bqkv,
                "wot": wotT,
                "bo": bo_,
                "kbias": kbias,
                "qmask": qmask,
            }
        )
    return in_maps


_NC_CACHE = {}


def _get_nc(with_bias=True):
    if with_bias not in _NC_CACHE:
        _NC_CACHE[with_bias] = build_nc(with_bias)
    return _NC_CACHE[with_bias]


def kernel(x, key_padding_mask, Wqkv_w, Wqkv_b, out_w, out_b):
    in_maps = prep_in_maps(x, key_padding_mask, Wqkv_w, Wqkv_b, out_w, out_b)
    with_bias = bool(np.any(np.asarray(Wqkv_b) != 0) or np.any(np.asarray(out_b) != 0))
    nc = _get_nc(with_bias)
    res = run_bass_kernel_spmd(nc, in_maps, core_ids=list(range(B)))
    out = np.stack([res.results[i]["out"] for i in range(B)], axis=0)
    return out.astype(np.float32)


if __name__ == "__main__":
    nc = build_nc(with_bias=False)
    print("build ok")
